# revision 29
# baseline (speedup 1.0000x reference)
"""Trainium2 Bass kernel for BQFeatureInjector (ball-query + top-k + MLP + masked max-pool, 7 stacks).

Strategy (per core, queries sharded 8-way, points replicated):
  z = -d2 via K=24 bf16 3-way-split PE matmul (fp32-accurate)
  top-32 per query via 4x (max8 / max_index / match_replace) on DVE
  thresholds tau_s = max(z_(k), -r_s^2) turn top-k-within-ball sets into value tests
  3-tier DRAM table [A1|A2] / [-BIG|A2] / [-BIG|-BIG], dma_gather(transpose) -> [128h, 2, q*32]
  segmented max-reduce -> M;  pooled = relu(M + (b1 - q@W1));  out += pooled @ W2 (PSUM-accumulated)
"""
import sys
import numpy as np

if '/opt/trn_rl_repo' not in sys.path:
    sys.path.insert(0, '/opt/trn_rl_repo')

import concourse.bass as bass  # noqa: E402
import concourse.bacc as bacc  # noqa: E402
import concourse.mybir as mybir  # noqa: E402
from concourse import tile  # noqa: E402
from concourse.mybir import AluOpType as alu  # noqa: E402
from concourse.mybir import AxisListType  # noqa: E402
from concourse import library_config  # noqa: E402

f32 = mybir.dt.float32
bf16 = mybir.dt.bfloat16
i16 = mybir.dt.int16
u16 = mybir.dt.uint16
u8 = mybir.dt.uint8

N_CORES = 8
NG, NS, NV = 4096, 4096, 8192
HID, BQH = 256, 128
R1SQ = float(np.float32(0.1) * np.float32(0.1))
R2SQ = float(np.float32(0.2) * np.float32(0.2))
BIG = 57344.0

PSETS = {'g': NG, 's': NS, 'v': NV}
# stack id -> (queryset, pointset); grouped by queryset for PSUM accumulation
QSTACKS = {'g': [(0, 'g'), (3, 's'), (4, 'v')],
           's': [(1, 's'), (5, 'g')],
           'v': [(2, 'v'), (6, 'g')]}


def build_nc(QSH):
    """QSH: dict queryset -> per-core query count. Returns compiled Bacc program."""
    nc = bacc.Bacc("TRN2", target_bir_lowering=False, debug=False, num_devices=N_CORES)

    pts = {k: nc.dram_tensor(f"pts_{k}", [PSETS[k], 3], f32, kind="ExternalInput") for k in 'gsv'}
    qry = {k: nc.dram_tensor(f"q_{k}", [QSH[k], 3], f32, kind="ExternalInput") for k in 'gsv'}
    tok = {k: nc.dram_tensor(f"tok_{k}", [QSH[k], HID], f32, kind="ExternalInput") for k in 'gsv'}
    W1 = nc.dram_tensor("W1", [7, 2, 3, BQH], f32, kind="ExternalInput")
    b1 = nc.dram_tensor("b1", [7, 2, BQH], f32, kind="ExternalInput")
    W2 = nc.dram_tensor("W2", [7, HID, HID], f32, kind="ExternalInput")
    b2 = nc.dram_tensor("b2", [7, HID], f32, kind="ExternalInput")
    out = {k: nc.dram_tensor(f"out_{k}", [QSH[k], HID], f32, kind="ExternalOutput") for k in 'gsv'}
    DBG = QSH.get('dbg', False)
    if DBG:
        dbg_v32 = nc.dram_tensor("dbg_v32", [128, 32], f32, kind="ExternalOutput")
        dbg_i32 = nc.dram_tensor("dbg_i32", [128, 32], f32, kind="ExternalOutput")
        dbg_fidx = nc.dram_tensor("dbg_fidx", [128, 32], f32, kind="ExternalOutput")
        dbg_M = nc.dram_tensor("dbg_M", [128, 256], f32, kind="ExternalOutput")
        dbg_pool = nc.dram_tensor("dbg_pool", [128, 256], f32, kind="ExternalOutput")
        dbg_z = nc.dram_tensor("dbg_z", [128, 4096], f32, kind="ExternalOutput")
        dbg_t2 = nc.dram_tensor("dbg_t2", [128, 1], f32, kind="ExternalOutput")
        dbg_m2 = nc.dram_tensor("dbg_m2", [128, 32], f32, kind="ExternalOutput")

    with tile.TileContext(nc) as tc:
        with (
            tc.tile_pool(name="persist", bufs=1) as pp,
            tc.tile_pool(name="zp", bufs=3, space="PSUM") as zp,
            tc.tile_pool(name="outp", bufs=2, space="PSUM") as op_,
            tc.tile_pool(name="tp", bufs=1, space="PSUM") as tp,
            tc.tile_pool(name="dtbl", bufs=1, space="DRAM") as dp,
            tc.tile_pool(name="didx", bufs=3, space="DRAM") as dip,
        ):
            sp = tc.alloc_tile_pool(name="prep", bufs=2)
            # ---------------- constants ----------------
            bigt = pp.tile([128, 256], bf16, tag="bigt", name="bigt")
            nc.vector.memset(bigt[:], -BIG)
            ident = pp.tile([128, 128], f32, tag="ident", name="ident")
            ones_t = pp.tile([128, 128], f32, tag="ones_t", name="ones_t")
            nc.vector.memset(ones_t[:], 1.0)
            nc.gpsimd.affine_select(ident[:], ones_t[:], pattern=[[-1, 128]], base=0,
                                    channel_multiplier=1, compare_op=alu.is_equal, fill=0.0)
            nc.gpsimd.load_library(library_config.mlp)
            dum = {}
            for k, Np in (('g', NG), ('v', NV)):
                d = pp.tile([128, 32], f32, tag=f"dum_{k}", name=f"dum_{k}")
                nc.vector.memset(d[:], float(2 * Np))
                dum[k] = d
            dum['s'] = dum['g']

            # ---------------- P^T loads (for A-table matmuls) ----------------
            PT = {}
            for k in 'gsv':
                t = sp.tile([3, PSETS[k]], f32, tag=f"PT_{k}", name=f"PT_{k}", bufs=1)
                nc.sync.dma_start(t[:], pts[k].ap().rearrange("n c -> c n"))
                PT[k] = t

            # ---------------- split-24 tables (p-side and q-side) ----------------
            def build_24(tc, src_dram, N, qside, stg, dst):
                """Build [24, N] bf16 split table via [128, A, 24] chunks + DRAM transpose bounce."""
                CH = 1024 if N % 1024 == 0 else 128
                A = CH // 128
                for base in range(0, N, CH):
                    pc = sp.tile([128, A, 3], f32, tag="b24_p", name="b24_p")
                    nc.sync.dma_start(pc[:], src_dram.ap()[base:base + CH, :]
                                      .rearrange("(a p) c -> p a c", p=128))
                    colst = sp.tile([128, A, 24], bf16, tag="b24_col", name="b24_col")
                    # norms from original coords: ((x^2+y^2)+z^2) in fp32
                    sq = sp.tile([128, A, 3], f32, tag="b24_sq", name="b24_sq")
                    nc.vector.tensor_tensor(sq[:], pc[:], pc[:], alu.mult)
                    nrm = sp.tile([128, A], f32, tag="b24_nrm", name="b24_nrm")
                    nc.vector.tensor_reduce(nrm[:], sq[:], AxisListType.X, alu.add)
                    if qside:
                        nc.vector.tensor_scalar(nrm[:], nrm[:], -1.0, None, alu.mult)
                    else:
                        # product side uses 2*p
                        nc.vector.tensor_scalar(pc[:], pc[:], 2.0, None, alu.mult)
                    # 3-way split of coords
                    hb = sp.tile([128, A, 3], bf16, tag="b24_hb", name="b24_hb")
                    nc.vector.tensor_copy(hb[:], pc[:])
                    hf = sp.tile([128, A, 3], f32, tag="b24_hf", name="b24_hf")
                    nc.vector.tensor_copy(hf[:], hb[:])
                    d1 = sp.tile([128, A, 3], f32, tag="b24_d1", name="b24_d1")
                    nc.vector.tensor_tensor(d1[:], pc[:], hf[:], alu.subtract)
                    mb = sp.tile([128, A, 3], bf16, tag="b24_mb", name="b24_mb")
                    nc.vector.tensor_copy(mb[:], d1[:])
                    mf = sp.tile([128, A, 3], f32, tag="b24_mf", name="b24_mf")
                    nc.vector.tensor_copy(mf[:], mb[:])
                    d2t = sp.tile([128, A, 3], f32, tag="b24_d2", name="b24_d2")
                    nc.vector.tensor_tensor(d2t[:], d1[:], mf[:], alu.subtract)
                    lb = sp.tile([128, A, 3], bf16, tag="b24_lb", name="b24_lb")
                    nc.vector.tensor_copy(lb[:], d2t[:])
                    # 3-way split of (signed) norm
                    nh = sp.tile([128, A], bf16, tag="b24_nh", name="b24_nh")
                    nc.vector.tensor_copy(nh[:], nrm[:])
                    nhf = sp.tile([128, A], f32, tag="b24_nhf", name="b24_nhf")
                    nc.vector.tensor_copy(nhf[:], nh[:])
                    nd1 = sp.tile([128, A], f32, tag="b24_nd1", name="b24_nd1")
                    nc.vector.tensor_tensor(nd1[:], nrm[:], nhf[:], alu.subtract)
                    nm = sp.tile([128, A], bf16, tag="b24_nm", name="b24_nm")
                    nc.vector.tensor_copy(nm[:], nd1[:])
                    nmf = sp.tile([128, A], f32, tag="b24_nmf", name="b24_nmf")
                    nc.vector.tensor_copy(nmf[:], nm[:])
                    nd2 = sp.tile([128, A], f32, tag="b24_nd2", name="b24_nd2")
                    nc.vector.tensor_tensor(nd2[:], nd1[:], nmf[:], alu.subtract)
                    nl = sp.tile([128, A], bf16, tag="b24_nl", name="b24_nl")
                    nc.vector.tensor_copy(nl[:], nd2[:])
                    # scatter into 24-col layout
                    v18 = colst[:, :, 0:18].rearrange("p a (c k) -> p a c k", k=6)
                    if qside:
                        # q rows per coord: qh,qh,qh,qm,qm,ql ; cols 18:21 = -qn splits ; 21:24 = -1
                        for kk in (0, 1, 2):
                            nc.vector.tensor_copy(v18[:, :, :, kk:kk + 1], hb[:].unsqueeze(3))
                        for kk in (3, 4):
                            nc.vector.tensor_copy(v18[:, :, :, kk:kk + 1], mb[:].unsqueeze(3))
                        nc.vector.tensor_copy(v18[:, :, :, 5:6], lb[:].unsqueeze(3))
                        nc.vector.tensor_copy(colst[:, :, 18:19], nh[:].unsqueeze(2))
                        nc.vector.tensor_copy(colst[:, :, 19:20], nm[:].unsqueeze(2))
                        nc.vector.tensor_copy(colst[:, :, 20:21], nl[:].unsqueeze(2))
                        nc.vector.memset(colst[:, :, 21:24], -1.0)
                    else:
                        # p rows per coord: 2ph,2pm,2pl,2ph,2pm,2ph ; 18:21 = 1 ; 21:24 = pn splits
                        for kk in (0, 3, 5):
                            nc.vector.tensor_copy(v18[:, :, :, kk:kk + 1], hb[:].unsqueeze(3))
                        for kk in (1, 4):
                            nc.vector.tensor_copy(v18[:, :, :, kk:kk + 1], mb[:].unsqueeze(3))
                        nc.vector.tensor_copy(v18[:, :, :, 2:3], lb[:].unsqueeze(3))
                        nc.vector.memset(colst[:, :, 18:21], 1.0)
                        nc.vector.tensor_copy(colst[:, :, 21:22], nh[:].unsqueeze(2))
                        nc.vector.tensor_copy(colst[:, :, 22:23], nm[:].unsqueeze(2))
                        nc.vector.tensor_copy(colst[:, :, 23:24], nl[:].unsqueeze(2))
                    nc.sync.dma_start(stg[base:base + CH, :]
                                      .rearrange("(a p) c -> p a c", p=128), colst[:])
                nc.sync.dma_start(dst[:], stg.rearrange("n r -> r n"))

            rhstab, q24 = {}, {}
            for k in 'gsv':
                stg = dp.tile([PSETS[k], 24], bf16, tag=f"stg24_{k}", name=f"stg24_{k}")
                rhstab[k] = pp.tile([24, PSETS[k]], bf16, tag=f"rhs_{k}", name=f"rhs_{k}")
                build_24(tc, pts[k], PSETS[k], False, stg, rhstab[k])
                stq = dp.tile([QSH[k], 24], bf16, tag=f"stg24q_{k}", name=f"stg24q_{k}")
                q24[k] = pp.tile([24, QSH[k]], bf16, tag=f"q24_{k}", name=f"q24_{k}")
                build_24(tc, qry[k], QSH[k], True, stq, q24[k])

            # ---------------- per-stack A tables / C / W2 / b1 ----------------
            ALL_STACKS = [(st, qs, ps) for qs, lst in QSTACKS.items() for st, ps in lst]
            tbl, C, W2sb = {}, {}, {}
            for st, qs, ps in ALL_STACKS:
                Np = PSETS[ps]
                R = 2 * Np + 128
                tb = dp.tile([R, 256], bf16, tag=f"tbl_{st}", name=f"tbl_{st}")
                tbl[st] = tb
                w1f = pp.tile([3, 256], f32, tag=f"w1f_{st}", name=f"w1f_{st}")
                nc.sync.dma_start(w1f[:].rearrange("c (s h) -> c s h", s=2),
                                  W1.ap()[st].rearrange("s c h -> c s h"))
                for base in range(0, Np, 1024):
                    stA = sp.tile([128, 8, 256], bf16, tag="stA", name="stA")
                    for c8 in range(8):
                        psA = zp.tile([128, 512], f32, tag="zpt", name="zpt")
                        nc.tensor.matmul(psA[:, 0:256], PT[ps][:, base + c8 * 128: base + (c8 + 1) * 128],
                                         w1f[:], start=True, stop=True)
                        nc.scalar.copy(stA[:, c8, :], psA[:, 0:256])
                    nc.sync.dma_start(tb[base:base + 1024, :]
                                      .rearrange("(a p) h -> p a h", p=128), stA[:])
                    nc.sync.dma_start(tb[Np + base:Np + base + 1024, 128:256]
                                      .rearrange("(a p) h -> p a h", p=128), stA[:, :, 128:256])
                # -BIG fills: tier2 first half + dummy rows
                nbk = Np // 128
                nc.sync.dma_start(tb[Np:2 * Np, 0:128].rearrange("(a p) h -> p a h", p=128),
                                  bigt[:, 0:128].unsqueeze(1).broadcast_to((128, nbk, 128)))
                nc.sync.dma_start(tb[2 * Np:2 * Np + 128, :], bigt[:])

                # C = b1 - q @ W1 in [h, 2, q] layout
                NQ = QSH[qs]
                Ct = pp.tile([128, 2, NQ], f32, tag=f"C_{st}", name=f"C_{st}")
                C[st] = Ct
                nqt = sp.tile([3, NQ], f32, tag="nqt", name="nqt")
                nc.sync.dma_start(nqt[:], qry[qs].ap().rearrange("n c -> c n"))
                nc.scalar.mul(nqt[:], nqt[:], -1.0)
                b1sb = sp.tile([128, 2], f32, tag="b1sb", name="b1sb")
                nc.sync.dma_start(b1sb[:], b1.ap()[st].rearrange("s h -> h s"))
                for sc in range(2):
                    for cb in range(0, NQ, 512):
                        cw = min(512, NQ - cb)
                        psC = zp.tile([128, 512], f32, tag="zpt", name="zpt")
                        nc.tensor.matmul(psC[:, 0:cw], w1f[:, sc * 128:(sc + 1) * 128],
                                         nqt[:, cb:cb + cw], start=True, stop=True)
                        nc.vector.tensor_scalar(Ct[:, sc, cb:cb + cw], psC[:, 0:cw],
                                                b1sb[:, sc:sc + 1], None, alu.add)
                # W2 -> bf16 [128, 2, 256]
                wst = sp.tile([128, 2, 256], f32, tag="w2st", name="w2st")
                nc.sync.dma_start(wst[:], W2.ap()[st].rearrange("(k p) o -> p k o", p=128))
                wsb = pp.tile([128, 2, 256], bf16, tag=f"w2sb_{st}", name=f"w2sb_{st}")
                nc.vector.tensor_copy(wsb[:], wst[:])
                W2sb[st] = wsb

            # b2 sums per queryset, [h(o-dim on partitions), 2]
            b2s = {}
            for qs, lst in QSTACKS.items():
                acc = pp.tile([128, 2], f32, tag=f"b2s_{qs}", name=f"b2s_{qs}")
                tmp = sp.tile([128, 2], f32, tag="b2tmp", name="b2tmp")
                first = True
                for st, ps in lst:
                    dst = acc if first else tmp
                    nc.sync.dma_start(dst[:], b2.ap()[st].rearrange("(k p) -> p k", p=128))
                    if not first:
                        nc.vector.tensor_tensor(acc[:], acc[:], tmp[:], alu.add)
                    first = False
                b2s[qs] = acc

            sp.release()
            # ---------------- main loop ----------------
            wp = tc.alloc_tile_pool(name="work", bufs=2)
            sp = tc.alloc_tile_pool(name="small", bufs=3)
            for qs in 'gsv':
                NQ = QSH[qs]
                stacks = QSTACKS[qs]
                for t in range(NQ // 128):
                    q0 = t * 128
                    po = [op_.tile([128, 128], f32, tag=f"po{mo}", name=f"po{mo}") for mo in range(2)]
                    for si, (st, ps) in enumerate(stacks):
                        Np = PSETS[ps]
                        zt = wp.tile([128, NV], f32, tag="zt", name="zt", bufs=QSH.get("ztb", 2))
                        for cb in range(0, Np, 512):
                            psz = zp.tile([128, 512], f32, tag="zpt", name="zpt")
                            nc.tensor.matmul(psz[:], q24[qs][:, q0:q0 + 128],
                                             rhstab[ps][:, cb:cb + 512], start=True, stop=True)
                            nc.scalar.copy(zt[:, cb:cb + 512], psz[:])
                        v32 = sp.tile([128, 32], f32, tag="v32", name="v32")
                        i32 = sp.tile([128, 32], u16, tag="i32", name="i32")
                        if QSH.get('no_sel'):
                            nc.vector.memset(v32[:], -1.0)
                            nc.vector.memset(i32[:], 0)
                        else:
                            # two-level top-32: per-128-chunk top-8 (depth<=7 verified on data),
                            # then 4 rounds on the candidate array; indices from pristine z
                            ncand = Np // 16
                            cand = sp.tile([128, NV // 16], f32, tag="cand", name="cand")
                            for cc in range(Np // 128):
                                nc.vector.max(cand[:, 8 * cc:8 * cc + 8],
                                              zt[:, 128 * cc:128 * cc + 128])
                            for r in range(4):
                                nc.vector.max(v32[:, 8 * r:8 * r + 8], cand[:, 0:ncand])
                                if r < 3:
                                    nc.vector.match_replace(cand[:, 0:ncand], v32[:, 8 * r:8 * r + 8],
                                                            cand[:, 0:ncand], -BIG)
                            for r in range(4):
                                nc.vector.max_index(i32[:, 8 * r:8 * r + 8], v32[:, 8 * r:8 * r + 8],
                                                    zt[:, 0:Np])
                        tau1 = sp.tile([128, 1], f32, tag="tau1", name="tau1")
                        nc.vector.tensor_scalar_max(tau1[:], v32[:, 15:16], -R1SQ)
                        tau2 = sp.tile([128, 1], f32, tag="tau2", name="tau2")
                        nc.vector.tensor_scalar_max(tau2[:], v32[:, 31:32], -R2SQ)
                        m1 = sp.tile([128, 32], f32, tag="m1", name="m1")
                        nc.vector.tensor_scalar(m1[:], v32[:], tau1[:], None, alu.is_ge)
                        m2 = sp.tile([128, 32], f32, tag="m2", name="m2")
                        nc.vector.tensor_scalar(m2[:], v32[:], tau2[:], None, alu.is_ge)
                        if DBG and qs == 'g' and t == 0 and si == 0:
                            nc.sync.dma_start(dbg_t2.ap(), tau2[:])
                            nc.sync.dma_start(dbg_m2.ap(), m2[:])
                        fidx = sp.tile([128, 32], f32, tag="fidx", name="fidx")
                        nc.vector.tensor_copy(fidx[:], i32[:])
                        # fidx += Np * (1 - m1);  invalid2 -> 2Np
                        mn = sp.tile([128, 32], f32, tag="mn", name="mn")
                        nc.vector.tensor_scalar(mn[:], m1[:], float(-Np), float(Np), alu.mult, alu.add)
                        fidxb = sp.tile([128, 32], f32, tag="fidxb", name="fidxb")
                        nc.vector.tensor_tensor(fidxb[:], fidx[:], mn[:], alu.add)
                        m2u = sp.tile([128, 32], u8, tag="m2u", name="m2u")
                        nc.vector.tensor_copy(m2u[:], m2[:])
                        fidx2 = sp.tile([128, 32], f32, tag="fidx2", name="fidx2")
                        nc.vector.select(fidx2[:], m2u[:], fidxb[:], dum[ps][:])
                        fx16 = sp.tile([128, 32], i16, tag="fx16", name="fx16")
                        nc.vector.tensor_copy(fx16[:], fidx2[:])
                        idxb = dip.tile([4096], i16, tag="idxb", name="idxb")
                        nc.sync.dma_start(idxb.rearrange("(p i) -> p i", p=128), fx16[:])
                        idxw = sp.tile([128, 256], i16, tag="idxw", name="idxw")
                        for g8 in range(8):
                            nc.sync.dma_start(idxw[16 * g8:16 * (g8 + 1), :],
                                              idxb.rearrange("(f p) -> p f", p=16))
                        G = wp.tile([128, 8, 2, 512], bf16, tag="G", name="G", bufs=QSH.get("gb", 1))
                        if QSH.get('no_gather'):
                            nc.vector.memset(G[:], 0.0)
                        else:
                            for gb in range(8):
                                nc.gpsimd.dma_gather(G[:, gb, :, :], tbl[st],
                                                     idxw[:, 32 * gb:32 * (gb + 1)],
                                                     num_idxs=512, num_idxs_reg=512,
                                                     elem_size=256, transpose=True)
                        M = sp.tile([128, 8, 2, 16], f32, tag="M", name="M")
                        Gv = G[:].rearrange("p g c (j i) -> p g c j i", i=32)
                        t16 = sp.tile([128, 8, 2, 16, 16], bf16, tag="t16", name="t16", bufs=QSH.get("t16b", 1))
                        nc.vector.tensor_tensor(t16[:], Gv[:, :, :, :, 0:16], Gv[:, :, :, :, 16:32], alu.max)
                        t8 = sp.tile([128, 8, 2, 16, 8], bf16, tag="t8", name="t8", bufs=1)
                        nc.vector.tensor_tensor(t8[:], t16[:, :, :, :, 0:8], t16[:, :, :, :, 8:16], alu.max)
                        t4 = sp.tile([128, 8, 2, 16, 4], bf16, tag="t4", name="t4", bufs=1)
                        nc.vector.tensor_tensor(t4[:], t8[:, :, :, :, 0:4], t8[:, :, :, :, 4:8], alu.max)
                        t2 = sp.tile([128, 8, 2, 16, 2], bf16, tag="t2", name="t2", bufs=1)
                        nc.vector.tensor_tensor(t2[:], t4[:, :, :, :, 0:2], t4[:, :, :, :, 2:4], alu.max)
                        nc.vector.tensor_tensor(M[:].unsqueeze(4), t2[:, :, :, :, 0:1],
                                                t2[:, :, :, :, 1:2], alu.max)
                        nc.vector.tensor_tensor(M[:], M[:],
                                                C[st][:, :, q0:q0 + 128]
                                                .rearrange("p c (g j) -> p g c j", g=8), alu.add)
                        poolb = sp.tile([128, 8, 2, 16], bf16, tag="poolb", name="poolb")
                        nc.vector.tensor_scalar_max(poolb[:], M[:], 0.0)
                        if DBG and qs == 'g' and t == 0 and si == 0:
                            nc.sync.dma_start(dbg_v32.ap(), v32[:])
                            i32f = sp.tile([128, 32], f32, tag="i32f", name="i32f")
                            nc.vector.tensor_copy(i32f[:], i32[:])
                            nc.sync.dma_start(dbg_i32.ap(), i32f[:])
                            nc.sync.dma_start(dbg_fidx.ap(), fidx2[:])
                            nc.sync.dma_start(dbg_M.ap(), M[:].rearrange("p g c j -> p (g c j)"))
                            pbf = sp.tile([128, 8, 2, 16], f32, tag="pbf", name="pbf")
                            nc.vector.tensor_copy(pbf[:], poolb[:])
                            nc.sync.dma_start(dbg_pool.ap(), pbf[:].rearrange("p g c j -> p (g c j)"))
                            nc.sync.dma_start(dbg_z.ap(), zt[:, 0:4096])
                        for ko in range(2):
                            for mo in range(2):
                                nc.tensor.matmul(po[mo][:], W2sb[st][:, ko, mo * 128:(mo + 1) * 128],
                                                 poolb[:, :, ko, :],
                                                 start=(si == 0 and ko == 0),
                                                 stop=(si == len(stacks) - 1 and ko == 1))
                    toks = sp.tile([128, 256], f32, tag="toks", name="toks")
                    nc.sync.dma_start(toks[:], tok[qs].ap()[q0:q0 + 128, :])
                    fin = sp.tile([128, 256], f32, tag="fin", name="fin")
                    for mo in range(2):
                        ob = sp.tile([128, 128], f32, tag="ob", name="ob")
                        nc.vector.tensor_scalar(ob[:], po[mo][:], b2s[qs][:, mo:mo + 1], None, alu.add)
                        ptp = tp.tile([128, 128], f32, tag="ptp", name="ptp")
                        nc.tensor.transpose(ptp[:], ob[:], ident[:])
                        nc.vector.tensor_tensor(fin[:, mo * 128:(mo + 1) * 128],
                                                toks[:, mo * 128:(mo + 1) * 128], ptp[:], alu.add)
                    nc.sync.dma_start(out[qs].ap()[q0:q0 + 128, :], fin[:])
            sp.release()
            wp.release()

    nc.compile()
    return nc


_NC_CACHE = {}


def _get_nc(QSH):
    key = tuple(sorted(QSH.items()))
    if key not in _NC_CACHE:
        _NC_CACHE[key] = build_nc(QSH)
    return _NC_CACHE[key]


def kernel(geometry_points, surface_points, volume_points,
           geo_tokens, surf_tokens, vol_tokens, W1, b1, W2, b2, _trace=False):
    from concourse.bass_utils import run_bass_kernel_spmd
    g = np.ascontiguousarray(np.asarray(geometry_points, np.float32)[0])
    s = np.ascontiguousarray(np.asarray(surface_points, np.float32)[0])
    v = np.ascontiguousarray(np.asarray(volume_points, np.float32)[0])
    tg = np.ascontiguousarray(np.asarray(geo_tokens, np.float32)[0])
    ts = np.ascontiguousarray(np.asarray(surf_tokens, np.float32)[0])
    tv = np.ascontiguousarray(np.asarray(vol_tokens, np.float32)[0])
    W1 = np.ascontiguousarray(np.asarray(W1, np.float32))
    b1 = np.ascontiguousarray(np.asarray(b1, np.float32))
    W2 = np.ascontiguousarray(np.asarray(W2, np.float32))
    b2 = np.ascontiguousarray(np.asarray(b2, np.float32))

    QSH = {'g': NG // N_CORES, 's': NS // N_CORES, 'v': NV // N_CORES, 'ztb': 1, 'gb': 2}
    nc = _get_nc(QSH)
    in_maps = []
    for c in range(N_CORES):
        sl = lambda a, n: np.ascontiguousarray(a[c * n:(c + 1) * n])
        in_maps.append({
            'pts_g': g, 'pts_s': s, 'pts_v': v,
            'q_g': sl(g, QSH['g']), 'q_s': sl(s, QSH['s']), 'q_v': sl(v, QSH['v']),
            'tok_g': sl(tg, QSH['g']), 'tok_s': sl(ts, QSH['s']), 'tok_v': sl(tv, QSH['v']),
            'W1': W1, 'b1': b1, 'W2': W2, 'b2': b2,
        })
    import time as _time
    _t0 = _time.time()
    res = run_bass_kernel_spmd(nc, in_maps, core_ids=list(range(N_CORES)), trace=_trace)
    kernel.last_spmd_wall_s = _time.time() - _t0
    og = np.concatenate([r['out_g'] for r in res.results], 0)[None]
    os_ = np.concatenate([r['out_s'] for r in res.results], 0)[None]
    ov = np.concatenate([r['out_v'] for r in res.results], 0)[None]
    kernel.last_exec_time_ns = res.exec_time_ns
    if kernel.last_exec_time_ns is None:
        kernel.last_exec_time_ns = int(kernel.last_spmd_wall_s * 1e9)
    return og, os_, ov


# revision 30
# speedup vs baseline: 1.0060x; 1.0060x over previous
"""Trainium2 Bass kernel for BQFeatureInjector (ball-query + top-k + MLP + masked max-pool, 7 stacks).

Strategy (per core, queries sharded 8-way, points replicated):
  z = -d2 via K=24 bf16 3-way-split PE matmul (fp32-accurate)
  top-32 per query via 4x (max8 / max_index / match_replace) on DVE
  thresholds tau_s = max(z_(k), -r_s^2) turn top-k-within-ball sets into value tests
  3-tier DRAM table [A1|A2] / [-BIG|A2] / [-BIG|-BIG], dma_gather(transpose) -> [128h, 2, q*32]
  segmented max-reduce -> M;  pooled = relu(M + (b1 - q@W1));  out += pooled @ W2 (PSUM-accumulated)
"""
import sys
import numpy as np

if '/opt/trn_rl_repo' not in sys.path:
    sys.path.insert(0, '/opt/trn_rl_repo')

import concourse.bass as bass  # noqa: E402
import concourse.bacc as bacc  # noqa: E402
import concourse.mybir as mybir  # noqa: E402
from concourse import tile  # noqa: E402
from concourse.mybir import AluOpType as alu  # noqa: E402
from concourse.mybir import AxisListType  # noqa: E402
from concourse import library_config  # noqa: E402

f32 = mybir.dt.float32
bf16 = mybir.dt.bfloat16
i16 = mybir.dt.int16
u16 = mybir.dt.uint16
u8 = mybir.dt.uint8

N_CORES = 8
NG, NS, NV = 4096, 4096, 8192
HID, BQH = 256, 128
R1SQ = float(np.float32(0.1) * np.float32(0.1))
R2SQ = float(np.float32(0.2) * np.float32(0.2))
BIG = 57344.0

PSETS = {'g': NG, 's': NS, 'v': NV}
# stack id -> (queryset, pointset); grouped by queryset for PSUM accumulation
QSTACKS = {'g': [(0, 'g'), (3, 's'), (4, 'v')],
           's': [(1, 's'), (5, 'g')],
           'v': [(2, 'v'), (6, 'g')]}


def build_nc(QSH):
    """QSH: dict queryset -> per-core query count. Returns compiled Bacc program."""
    nc = bacc.Bacc("TRN2", target_bir_lowering=False, debug=False, num_devices=N_CORES)

    pts = {k: nc.dram_tensor(f"pts_{k}", [PSETS[k], 3], f32, kind="ExternalInput") for k in 'gsv'}
    qry = {k: nc.dram_tensor(f"q_{k}", [QSH[k], 3], f32, kind="ExternalInput") for k in 'gsv'}
    tok = {k: nc.dram_tensor(f"tok_{k}", [QSH[k], HID], f32, kind="ExternalInput") for k in 'gsv'}
    W1 = nc.dram_tensor("W1", [7, 2, 3, BQH], f32, kind="ExternalInput")
    b1 = nc.dram_tensor("b1", [7, 2, BQH], f32, kind="ExternalInput")
    W2 = nc.dram_tensor("W2", [7, HID, HID], f32, kind="ExternalInput")
    b2 = nc.dram_tensor("b2", [7, HID], f32, kind="ExternalInput")
    out = {k: nc.dram_tensor(f"out_{k}", [QSH[k], HID], f32, kind="ExternalOutput") for k in 'gsv'}
    DBG = QSH.get('dbg', False)
    if DBG:
        dbg_v32 = nc.dram_tensor("dbg_v32", [128, 32], f32, kind="ExternalOutput")
        dbg_i32 = nc.dram_tensor("dbg_i32", [128, 32], f32, kind="ExternalOutput")
        dbg_fidx = nc.dram_tensor("dbg_fidx", [128, 32], f32, kind="ExternalOutput")
        dbg_M = nc.dram_tensor("dbg_M", [128, 256], f32, kind="ExternalOutput")
        dbg_pool = nc.dram_tensor("dbg_pool", [128, 256], f32, kind="ExternalOutput")
        dbg_z = nc.dram_tensor("dbg_z", [128, 4096], f32, kind="ExternalOutput")
        dbg_t2 = nc.dram_tensor("dbg_t2", [128, 1], f32, kind="ExternalOutput")
        dbg_m2 = nc.dram_tensor("dbg_m2", [128, 32], f32, kind="ExternalOutput")

    with tile.TileContext(nc) as tc:
        with (
            tc.tile_pool(name="persist", bufs=1) as pp,
            tc.tile_pool(name="zp", bufs=3, space="PSUM") as zp,
            tc.tile_pool(name="outp", bufs=2, space="PSUM") as op_,
            tc.tile_pool(name="tp", bufs=1, space="PSUM") as tp,
            tc.tile_pool(name="dtbl", bufs=1, space="DRAM") as dp,
            tc.tile_pool(name="didx", bufs=3, space="DRAM") as dip,
        ):
            sp = tc.alloc_tile_pool(name="prep", bufs=2)
            # ---------------- constants ----------------
            bigt = pp.tile([128, 256], bf16, tag="bigt", name="bigt")
            nc.vector.memset(bigt[:], -BIG)
            ident = pp.tile([128, 128], f32, tag="ident", name="ident")
            ones_t = pp.tile([128, 128], f32, tag="ones_t", name="ones_t")
            nc.vector.memset(ones_t[:], 1.0)
            nc.gpsimd.affine_select(ident[:], ones_t[:], pattern=[[-1, 128]], base=0,
                                    channel_multiplier=1, compare_op=alu.is_equal, fill=0.0)
            nc.gpsimd.load_library(library_config.mlp)
            dum = {}
            for k, Np in (('g', NG), ('v', NV)):
                d = pp.tile([128, 32], f32, tag=f"dum_{k}", name=f"dum_{k}")
                nc.vector.memset(d[:], float(2 * Np))
                dum[k] = d
            dum['s'] = dum['g']

            # ---------------- P^T loads (for A-table matmuls) ----------------
            PT = {}
            for k in 'gsv':
                t = sp.tile([3, PSETS[k]], f32, tag=f"PT_{k}", name=f"PT_{k}", bufs=1)
                nc.sync.dma_start(t[:], pts[k].ap().rearrange("n c -> c n"))
                PT[k] = t

            # ---------------- split-24 tables (p-side and q-side) ----------------
            def build_24(tc, src_dram, N, qside, stg, dst):
                """Build [24, N] bf16 split table via [128, A, 24] chunks + DRAM transpose bounce."""
                CH = 1024 if N % 1024 == 0 else 128
                A = CH // 128
                for base in range(0, N, CH):
                    pc = sp.tile([128, A, 3], f32, tag="b24_p", name="b24_p")
                    nc.sync.dma_start(pc[:], src_dram.ap()[base:base + CH, :]
                                      .rearrange("(a p) c -> p a c", p=128))
                    colst = sp.tile([128, A, 24], bf16, tag="b24_col", name="b24_col")
                    # norms from original coords: ((x^2+y^2)+z^2) in fp32
                    sq = sp.tile([128, A, 3], f32, tag="b24_sq", name="b24_sq")
                    nc.vector.tensor_tensor(sq[:], pc[:], pc[:], alu.mult)
                    nrm = sp.tile([128, A], f32, tag="b24_nrm", name="b24_nrm")
                    nc.vector.tensor_reduce(nrm[:], sq[:], AxisListType.X, alu.add)
                    if qside:
                        nc.vector.tensor_scalar(nrm[:], nrm[:], -1.0, None, alu.mult)
                    else:
                        # product side uses 2*p
                        nc.vector.tensor_scalar(pc[:], pc[:], 2.0, None, alu.mult)
                    # 3-way split of coords
                    hb = sp.tile([128, A, 3], bf16, tag="b24_hb", name="b24_hb")
                    nc.vector.tensor_copy(hb[:], pc[:])
                    hf = sp.tile([128, A, 3], f32, tag="b24_hf", name="b24_hf")
                    nc.vector.tensor_copy(hf[:], hb[:])
                    d1 = sp.tile([128, A, 3], f32, tag="b24_d1", name="b24_d1")
                    nc.vector.tensor_tensor(d1[:], pc[:], hf[:], alu.subtract)
                    mb = sp.tile([128, A, 3], bf16, tag="b24_mb", name="b24_mb")
                    nc.vector.tensor_copy(mb[:], d1[:])
                    mf = sp.tile([128, A, 3], f32, tag="b24_mf", name="b24_mf")
                    nc.vector.tensor_copy(mf[:], mb[:])
                    d2t = sp.tile([128, A, 3], f32, tag="b24_d2", name="b24_d2")
                    nc.vector.tensor_tensor(d2t[:], d1[:], mf[:], alu.subtract)
                    lb = sp.tile([128, A, 3], bf16, tag="b24_lb", name="b24_lb")
                    nc.vector.tensor_copy(lb[:], d2t[:])
                    # 3-way split of (signed) norm
                    nh = sp.tile([128, A], bf16, tag="b24_nh", name="b24_nh")
                    nc.vector.tensor_copy(nh[:], nrm[:])
                    nhf = sp.tile([128, A], f32, tag="b24_nhf", name="b24_nhf")
                    nc.vector.tensor_copy(nhf[:], nh[:])
                    nd1 = sp.tile([128, A], f32, tag="b24_nd1", name="b24_nd1")
                    nc.vector.tensor_tensor(nd1[:], nrm[:], nhf[:], alu.subtract)
                    nm = sp.tile([128, A], bf16, tag="b24_nm", name="b24_nm")
                    nc.vector.tensor_copy(nm[:], nd1[:])
                    nmf = sp.tile([128, A], f32, tag="b24_nmf", name="b24_nmf")
                    nc.vector.tensor_copy(nmf[:], nm[:])
                    nd2 = sp.tile([128, A], f32, tag="b24_nd2", name="b24_nd2")
                    nc.vector.tensor_tensor(nd2[:], nd1[:], nmf[:], alu.subtract)
                    nl = sp.tile([128, A], bf16, tag="b24_nl", name="b24_nl")
                    nc.vector.tensor_copy(nl[:], nd2[:])
                    # scatter into 24-col layout
                    v18 = colst[:, :, 0:18].rearrange("p a (c k) -> p a c k", k=6)
                    if qside:
                        # q rows per coord: qh,qh,qh,qm,qm,ql ; cols 18:21 = -qn splits ; 21:24 = -1
                        for kk in (0, 1, 2):
                            nc.vector.tensor_copy(v18[:, :, :, kk:kk + 1], hb[:].unsqueeze(3))
                        for kk in (3, 4):
                            nc.vector.tensor_copy(v18[:, :, :, kk:kk + 1], mb[:].unsqueeze(3))
                        nc.vector.tensor_copy(v18[:, :, :, 5:6], lb[:].unsqueeze(3))
                        nc.vector.tensor_copy(colst[:, :, 18:19], nh[:].unsqueeze(2))
                        nc.vector.tensor_copy(colst[:, :, 19:20], nm[:].unsqueeze(2))
                        nc.vector.tensor_copy(colst[:, :, 20:21], nl[:].unsqueeze(2))
                        nc.vector.memset(colst[:, :, 21:24], -1.0)
                    else:
                        # p rows per coord: 2ph,2pm,2pl,2ph,2pm,2ph ; 18:21 = 1 ; 21:24 = pn splits
                        for kk in (0, 3, 5):
                            nc.vector.tensor_copy(v18[:, :, :, kk:kk + 1], hb[:].unsqueeze(3))
                        for kk in (1, 4):
                            nc.vector.tensor_copy(v18[:, :, :, kk:kk + 1], mb[:].unsqueeze(3))
                        nc.vector.tensor_copy(v18[:, :, :, 2:3], lb[:].unsqueeze(3))
                        nc.vector.memset(colst[:, :, 18:21], 1.0)
                        nc.vector.tensor_copy(colst[:, :, 21:22], nh[:].unsqueeze(2))
                        nc.vector.tensor_copy(colst[:, :, 22:23], nm[:].unsqueeze(2))
                        nc.vector.tensor_copy(colst[:, :, 23:24], nl[:].unsqueeze(2))
                    nc.sync.dma_start(stg[base:base + CH, :]
                                      .rearrange("(a p) c -> p a c", p=128), colst[:])
                nc.sync.dma_start(dst[:], stg.rearrange("n r -> r n"))

            rhstab, q24 = {}, {}
            for k in 'gsv':
                stg = dp.tile([PSETS[k], 24], bf16, tag=f"stg24_{k}", name=f"stg24_{k}")
                rhstab[k] = pp.tile([24, PSETS[k]], bf16, tag=f"rhs_{k}", name=f"rhs_{k}")
                build_24(tc, pts[k], PSETS[k], False, stg, rhstab[k])
                stq = dp.tile([QSH[k], 24], bf16, tag=f"stg24q_{k}", name=f"stg24q_{k}")
                q24[k] = pp.tile([24, QSH[k]], bf16, tag=f"q24_{k}", name=f"q24_{k}")
                build_24(tc, qry[k], QSH[k], True, stq, q24[k])

            # ---------------- per-stack A tables / C / W2 / b1 ----------------
            ALL_STACKS = [(st, qs, ps) for qs, lst in QSTACKS.items() for st, ps in lst]
            tbl, C, W2sb = {}, {}, {}
            for st, qs, ps in ALL_STACKS:
                Np = PSETS[ps]
                R = 2 * Np + 128
                tb = dp.tile([R, 256], bf16, tag=f"tbl_{st}", name=f"tbl_{st}")
                tbl[st] = tb
                w1f = pp.tile([3, 256], f32, tag=f"w1f_{st}", name=f"w1f_{st}")
                nc.sync.dma_start(w1f[:].rearrange("c (s h) -> c s h", s=2),
                                  W1.ap()[st].rearrange("s c h -> c s h"))
                for base in range(0, Np, 1024):
                    stA = sp.tile([128, 8, 256], bf16, tag="stA", name="stA")
                    for c8 in range(8):
                        psA = zp.tile([128, 512], f32, tag="zpt", name="zpt")
                        nc.tensor.matmul(psA[:, 0:256], PT[ps][:, base + c8 * 128: base + (c8 + 1) * 128],
                                         w1f[:], start=True, stop=True)
                        nc.scalar.copy(stA[:, c8, :], psA[:, 0:256])
                    nc.sync.dma_start(tb[base:base + 1024, :]
                                      .rearrange("(a p) h -> p a h", p=128), stA[:])
                    nc.sync.dma_start(tb[Np + base:Np + base + 1024, 128:256]
                                      .rearrange("(a p) h -> p a h", p=128), stA[:, :, 128:256])
                # -BIG fills: tier2 first half + dummy rows
                nbk = Np // 128
                nc.sync.dma_start(tb[Np:2 * Np, 0:128].rearrange("(a p) h -> p a h", p=128),
                                  bigt[:, 0:128].unsqueeze(1).broadcast_to((128, nbk, 128)))
                nc.sync.dma_start(tb[2 * Np:2 * Np + 128, :], bigt[:])

                # C = b1 - q @ W1 in [h, 2, q] layout
                NQ = QSH[qs]
                Ct = pp.tile([128, 2, NQ], f32, tag=f"C_{st}", name=f"C_{st}")
                C[st] = Ct
                nqt = sp.tile([3, NQ], f32, tag="nqt", name="nqt")
                nc.sync.dma_start(nqt[:], qry[qs].ap().rearrange("n c -> c n"))
                nc.scalar.mul(nqt[:], nqt[:], -1.0)
                b1sb = sp.tile([128, 2], f32, tag="b1sb", name="b1sb")
                nc.sync.dma_start(b1sb[:], b1.ap()[st].rearrange("s h -> h s"))
                for sc in range(2):
                    for cb in range(0, NQ, 512):
                        cw = min(512, NQ - cb)
                        psC = zp.tile([128, 512], f32, tag="zpt", name="zpt")
                        nc.tensor.matmul(psC[:, 0:cw], w1f[:, sc * 128:(sc + 1) * 128],
                                         nqt[:, cb:cb + cw], start=True, stop=True)
                        nc.vector.tensor_scalar(Ct[:, sc, cb:cb + cw], psC[:, 0:cw],
                                                b1sb[:, sc:sc + 1], None, alu.add)
                # W2 -> bf16 [128, 2, 256]
                wst = sp.tile([128, 2, 256], f32, tag="w2st", name="w2st")
                nc.sync.dma_start(wst[:], W2.ap()[st].rearrange("(k p) o -> p k o", p=128))
                wsb = pp.tile([128, 2, 256], bf16, tag=f"w2sb_{st}", name=f"w2sb_{st}")
                nc.vector.tensor_copy(wsb[:], wst[:])
                W2sb[st] = wsb

            # b2 sums per queryset, [h(o-dim on partitions), 2]
            b2s = {}
            for qs, lst in QSTACKS.items():
                acc = pp.tile([128, 2], f32, tag=f"b2s_{qs}", name=f"b2s_{qs}")
                tmp = sp.tile([128, 2], f32, tag="b2tmp", name="b2tmp")
                first = True
                for st, ps in lst:
                    dst = acc if first else tmp
                    nc.sync.dma_start(dst[:], b2.ap()[st].rearrange("(k p) -> p k", p=128))
                    if not first:
                        nc.vector.tensor_tensor(acc[:], acc[:], tmp[:], alu.add)
                    first = False
                b2s[qs] = acc

            sp.release()
            # ---------------- main loop ----------------
            wp = tc.alloc_tile_pool(name="work", bufs=2)
            sp = tc.alloc_tile_pool(name="small", bufs=QSH.get("smb", 3))
            for qs in 'gsv':
                NQ = QSH[qs]
                stacks = QSTACKS[qs]
                for t in range(NQ // 128):
                    q0 = t * 128
                    po = [op_.tile([128, 128], f32, tag=f"po{mo}", name=f"po{mo}") for mo in range(2)]
                    for si, (st, ps) in enumerate(stacks):
                        Np = PSETS[ps]
                        zt = wp.tile([128, NV], f32, tag="zt", name="zt", bufs=QSH.get("ztb", 2))
                        for cb in range(0, Np, 512):
                            psz = zp.tile([128, 512], f32, tag="zpt", name="zpt")
                            nc.tensor.matmul(psz[:], q24[qs][:, q0:q0 + 128],
                                             rhstab[ps][:, cb:cb + 512], start=True, stop=True)
                            nc.scalar.copy(zt[:, cb:cb + 512], psz[:])
                        v32 = sp.tile([128, 32], f32, tag="v32", name="v32")
                        i32 = sp.tile([128, 32], u16, tag="i32", name="i32")
                        if QSH.get('no_sel'):
                            nc.vector.memset(v32[:], -1.0)
                            nc.vector.memset(i32[:], 0)
                        else:
                            # two-level top-32: per-128-chunk top-8 (depth<=7 verified on data),
                            # then 4 rounds on the candidate array; indices from pristine z
                            ncand = Np // 16
                            cand = sp.tile([128, NV // 16], f32, tag="cand", name="cand")
                            for cc in range(Np // 128):
                                nc.vector.max(cand[:, 8 * cc:8 * cc + 8],
                                              zt[:, 128 * cc:128 * cc + 128])
                            for r in range(4):
                                nc.vector.max(v32[:, 8 * r:8 * r + 8], cand[:, 0:ncand])
                                if r < 3:
                                    nc.vector.match_replace(cand[:, 0:ncand], v32[:, 8 * r:8 * r + 8],
                                                            cand[:, 0:ncand], -BIG)
                            for r in range(4):
                                nc.vector.max_index(i32[:, 8 * r:8 * r + 8], v32[:, 8 * r:8 * r + 8],
                                                    zt[:, 0:Np])
                        tau1 = sp.tile([128, 1], f32, tag="tau1", name="tau1")
                        nc.vector.tensor_scalar_max(tau1[:], v32[:, 15:16], -R1SQ)
                        tau2 = sp.tile([128, 1], f32, tag="tau2", name="tau2")
                        nc.vector.tensor_scalar_max(tau2[:], v32[:, 31:32], -R2SQ)
                        m1 = sp.tile([128, 32], f32, tag="m1", name="m1")
                        nc.vector.tensor_scalar(m1[:], v32[:], tau1[:], None, alu.is_ge)
                        m2 = sp.tile([128, 32], f32, tag="m2", name="m2")
                        nc.vector.tensor_scalar(m2[:], v32[:], tau2[:], None, alu.is_ge)
                        if DBG and qs == 'g' and t == 0 and si == 0:
                            nc.sync.dma_start(dbg_t2.ap(), tau2[:])
                            nc.sync.dma_start(dbg_m2.ap(), m2[:])
                        fidx = sp.tile([128, 32], f32, tag="fidx", name="fidx")
                        nc.vector.tensor_copy(fidx[:], i32[:])
                        # fidx += Np * (1 - m1);  invalid2 -> 2Np
                        mn = sp.tile([128, 32], f32, tag="mn", name="mn")
                        nc.vector.tensor_scalar(mn[:], m1[:], float(-Np), float(Np), alu.mult, alu.add)
                        fidxb = sp.tile([128, 32], f32, tag="fidxb", name="fidxb")
                        nc.vector.tensor_tensor(fidxb[:], fidx[:], mn[:], alu.add)
                        m2u = sp.tile([128, 32], u8, tag="m2u", name="m2u")
                        nc.vector.tensor_copy(m2u[:], m2[:])
                        fidx2 = sp.tile([128, 32], f32, tag="fidx2", name="fidx2")
                        nc.vector.select(fidx2[:], m2u[:], fidxb[:], dum[ps][:])
                        fx16 = sp.tile([128, 32], i16, tag="fx16", name="fx16")
                        nc.vector.tensor_copy(fx16[:], fidx2[:])
                        idxb = dip.tile([4096], i16, tag="idxb", name="idxb")
                        nc.sync.dma_start(idxb.rearrange("(p i) -> p i", p=128), fx16[:])
                        idxw = sp.tile([128, 256], i16, tag="idxw", name="idxw")
                        for g8 in range(8):
                            nc.sync.dma_start(idxw[16 * g8:16 * (g8 + 1), :],
                                              idxb.rearrange("(f p) -> p f", p=16))
                        G = wp.tile([128, 8, 2, 512], bf16, tag="G", name="G", bufs=QSH.get("gb", 1))
                        if QSH.get('no_gather'):
                            nc.vector.memset(G[:], 0.0)
                        else:
                            for gb in range(8):
                                nc.gpsimd.dma_gather(G[:, gb, :, :], tbl[st],
                                                     idxw[:, 32 * gb:32 * (gb + 1)],
                                                     num_idxs=512, num_idxs_reg=512,
                                                     elem_size=256, transpose=True)
                        M = sp.tile([128, 8, 2, 16], f32, tag="M", name="M")
                        Gv = G[:].rearrange("p g c (j i) -> p g c j i", i=32)
                        t16 = sp.tile([128, 8, 2, 16, 16], bf16, tag="t16", name="t16", bufs=QSH.get("t16b", 1))
                        nc.vector.tensor_tensor(t16[:], Gv[:, :, :, :, 0:16], Gv[:, :, :, :, 16:32], alu.max)
                        t8 = sp.tile([128, 8, 2, 16, 8], bf16, tag="t8", name="t8", bufs=1)
                        nc.vector.tensor_tensor(t8[:], t16[:, :, :, :, 0:8], t16[:, :, :, :, 8:16], alu.max)
                        t4 = sp.tile([128, 8, 2, 16, 4], bf16, tag="t4", name="t4", bufs=1)
                        nc.vector.tensor_tensor(t4[:], t8[:, :, :, :, 0:4], t8[:, :, :, :, 4:8], alu.max)
                        t2 = sp.tile([128, 8, 2, 16, 2], bf16, tag="t2", name="t2", bufs=1)
                        nc.vector.tensor_tensor(t2[:], t4[:, :, :, :, 0:2], t4[:, :, :, :, 2:4], alu.max)
                        nc.vector.tensor_tensor(M[:].unsqueeze(4), t2[:, :, :, :, 0:1],
                                                t2[:, :, :, :, 1:2], alu.max)
                        nc.vector.tensor_tensor(M[:], M[:],
                                                C[st][:, :, q0:q0 + 128]
                                                .rearrange("p c (g j) -> p g c j", g=8), alu.add)
                        poolb = sp.tile([128, 8, 2, 16], bf16, tag="poolb", name="poolb")
                        nc.vector.tensor_scalar_max(poolb[:], M[:], 0.0)
                        if DBG and qs == 'g' and t == 0 and si == 0:
                            nc.sync.dma_start(dbg_v32.ap(), v32[:])
                            i32f = sp.tile([128, 32], f32, tag="i32f", name="i32f")
                            nc.vector.tensor_copy(i32f[:], i32[:])
                            nc.sync.dma_start(dbg_i32.ap(), i32f[:])
                            nc.sync.dma_start(dbg_fidx.ap(), fidx2[:])
                            nc.sync.dma_start(dbg_M.ap(), M[:].rearrange("p g c j -> p (g c j)"))
                            pbf = sp.tile([128, 8, 2, 16], f32, tag="pbf", name="pbf")
                            nc.vector.tensor_copy(pbf[:], poolb[:])
                            nc.sync.dma_start(dbg_pool.ap(), pbf[:].rearrange("p g c j -> p (g c j)"))
                            nc.sync.dma_start(dbg_z.ap(), zt[:, 0:4096])
                        for ko in range(2):
                            for mo in range(2):
                                nc.tensor.matmul(po[mo][:], W2sb[st][:, ko, mo * 128:(mo + 1) * 128],
                                                 poolb[:, :, ko, :],
                                                 start=(si == 0 and ko == 0),
                                                 stop=(si == len(stacks) - 1 and ko == 1))
                    toks = sp.tile([128, 256], f32, tag="toks", name="toks")
                    nc.sync.dma_start(toks[:], tok[qs].ap()[q0:q0 + 128, :])
                    fin = sp.tile([128, 256], f32, tag="fin", name="fin")
                    for mo in range(2):
                        ob = sp.tile([128, 128], f32, tag="ob", name="ob")
                        nc.vector.tensor_scalar(ob[:], po[mo][:], b2s[qs][:, mo:mo + 1], None, alu.add)
                        ptp = tp.tile([128, 128], f32, tag="ptp", name="ptp")
                        nc.tensor.transpose(ptp[:], ob[:], ident[:])
                        nc.vector.tensor_tensor(fin[:, mo * 128:(mo + 1) * 128],
                                                toks[:, mo * 128:(mo + 1) * 128], ptp[:], alu.add)
                    nc.sync.dma_start(out[qs].ap()[q0:q0 + 128, :], fin[:])
            sp.release()
            wp.release()

    nc.compile()
    return nc


_NC_CACHE = {}


def _get_nc(QSH):
    key = tuple(sorted(QSH.items()))
    if key not in _NC_CACHE:
        _NC_CACHE[key] = build_nc(QSH)
    return _NC_CACHE[key]


def kernel(geometry_points, surface_points, volume_points,
           geo_tokens, surf_tokens, vol_tokens, W1, b1, W2, b2, _trace=False):
    from concourse.bass_utils import run_bass_kernel_spmd
    g = np.ascontiguousarray(np.asarray(geometry_points, np.float32)[0])
    s = np.ascontiguousarray(np.asarray(surface_points, np.float32)[0])
    v = np.ascontiguousarray(np.asarray(volume_points, np.float32)[0])
    tg = np.ascontiguousarray(np.asarray(geo_tokens, np.float32)[0])
    ts = np.ascontiguousarray(np.asarray(surf_tokens, np.float32)[0])
    tv = np.ascontiguousarray(np.asarray(vol_tokens, np.float32)[0])
    W1 = np.ascontiguousarray(np.asarray(W1, np.float32))
    b1 = np.ascontiguousarray(np.asarray(b1, np.float32))
    W2 = np.ascontiguousarray(np.asarray(W2, np.float32))
    b2 = np.ascontiguousarray(np.asarray(b2, np.float32))

    QSH = {'g': NG // N_CORES, 's': NS // N_CORES, 'v': NV // N_CORES, 'ztb': 1, 'gb': 2}
    nc = _get_nc(QSH)
    in_maps = []
    for c in range(N_CORES):
        sl = lambda a, n: np.ascontiguousarray(a[c * n:(c + 1) * n])
        in_maps.append({
            'pts_g': g, 'pts_s': s, 'pts_v': v,
            'q_g': sl(g, QSH['g']), 'q_s': sl(s, QSH['s']), 'q_v': sl(v, QSH['v']),
            'tok_g': sl(tg, QSH['g']), 'tok_s': sl(ts, QSH['s']), 'tok_v': sl(tv, QSH['v']),
            'W1': W1, 'b1': b1, 'W2': W2, 'b2': b2,
        })
    import time as _time
    _t0 = _time.time()
    res = run_bass_kernel_spmd(nc, in_maps, core_ids=list(range(N_CORES)), trace=_trace)
    kernel.last_spmd_wall_s = _time.time() - _t0
    og = np.concatenate([r['out_g'] for r in res.results], 0)[None]
    os_ = np.concatenate([r['out_s'] for r in res.results], 0)[None]
    ov = np.concatenate([r['out_v'] for r in res.results], 0)[None]
    kernel.last_exec_time_ns = res.exec_time_ns
    if kernel.last_exec_time_ns is None:
        kernel.last_exec_time_ns = int(kernel.last_spmd_wall_s * 1e9)
    return og, os_, ov
